# revision 6
# baseline (speedup 1.0000x reference)
"""Trainium2 Bass kernel for causal self-attention with RoPE (Megatron-style
head-parallel over 8 NeuronCores).

Sharding: 16 heads / 8 cores = 2 heads per core. Wqkv is split column-wise by
head (each core computes q/k/v for its 2 heads for the full batch); attention
is embarrassingly parallel over (batch, head); the output projection is
row-parallel with the partial contraction exchanged via two AllToAlls (one per
local head) so that core r ends up owning output rows [r*512, (r+1)*512) of
the flattened [4096, 2048] output, which the host concatenates.

Key scheduling ideas vs the naive version:
 - all inputs are pre-converted to bf16 on the host (halves HBM traffic);
   first x chunk + first weight pieces ride separate DMA rings so the PE
   starts within a few us.
 - attention loops local-head OUTERMOST: after head 0 of every (b, tq-chunk)
   is done, its AllToAll launches and hides under head 1's compute; the
   second AllToAll hides under projection pass 1 (even global heads).
 - softmax normalization happens at the DESTINATION core: each a2a slot
   carries 128 unnormalized y rows + 1 denominator row; the receiver does
   reciprocal + partition-broadcast + per-head scaling before the projection.
 - causal structure is exploited at 128-column granularity: diagonal key
   blocks only compute score/av/den columns c >= 128*m (ascending-j PSUM
   accumulation), and the triangular mask shrinks to one [128,128] multiply.
 - the projection contracts in two passes (even heads, then odd heads after
   a2a#1) with bf16 partials; bias is folded into pass 1.

All matmuls run in bf16 with fp32 PSUM accumulation. Softmax skips the
max-subtraction (scores are O(+-10) here, exp stays in range). RoPE's
rotate-half runs as a constant +-1 permutation matmul on the PE.
"""

import sys

if "/opt/trn_rl_repo" not in sys.path:
    sys.path.insert(0, "/opt/trn_rl_repo")

import ml_dtypes
import numpy as np

import concourse.bacc as bacc
import concourse.bass as bass
import concourse.mybir as mybir
import concourse.tile as tile
from concourse.bass_utils import run_bass_kernel_spmd

B, T, C, H, D = 4, 1024, 2048, 16, 128
TQ = B * T           # 4096 flattened tokens
NCORES = 8
HPC = H // NCORES    # heads per core = 2
FQK = 4 * D          # 512 qkT feature rows per core (qa, qb, ka, kb)
FV = HPC * D         # 256 v feature cols per core
ROWS = TQ // NCORES  # 512 output rows per core
NCT = C // 128       # 16 contraction tiles
SCALE = 1.0 / float(np.sqrt(D))

F32 = mybir.dt.float32
BF16 = mybir.dt.bfloat16

_CACHE = {}


def _build_program():
    nc = bacc.Bacc(
        "TRN2",
        target_bir_lowering=False,
        debug=False,
        enable_asserts=False,
        num_devices=NCORES,
    )

    # ---- I/O (all big tensors pre-converted to bf16 on the host) -------
    xT = nc.dram_tensor("xT", [C, TQ], BF16, kind="ExternalInput")
    wqk = nc.dram_tensor("wqk", [C, FQK], BF16, kind="ExternalInput")
    wv = nc.dram_tensor("wv", [C, FV], BF16, kind="ExternalInput")
    bqk = nc.dram_tensor("bqk", [128, 4], F32, kind="ExternalInput")
    bv = nc.dram_tensor("bv", [128, FV], F32, kind="ExternalInput")
    wproj = nc.dram_tensor("wproj", [C, C], BF16, kind="ExternalInput")
    bproj = nc.dram_tensor("bproj", [128, C], BF16, kind="ExternalInput")
    cosd = nc.dram_tensor("cosd", [128, T], BF16, kind="ExternalInput")
    sind = nc.dram_tensor("sind", [128, T], BF16, kind="ExternalInput")
    rmat = nc.dram_tensor("rmat", [128, 128], BF16, kind="ExternalInput")
    out = nc.dram_tensor("out", [ROWS, C], F32, kind="ExternalOutput")

    NT = TQ // 512  # 8 token chunks of 512
    Exp = mybir.ActivationFunctionType.Exp
    add = mybir.AluOpType.add
    mult = mybir.AluOpType.mult

    with tile.TileContext(nc) as tc:
        with (
            tc.tile_pool(name="const", bufs=1) as cpool,
            tc.tile_pool(name="resident", bufs=1) as rpool,
            tc.tile_pool(name="work", bufs=2) as wpool,
            tc.tile_pool(name="att", bufs=2) as apool,
            tc.tile_pool(name="psMM", bufs=2, space="PSUM") as psMM,
            tc.tile_pool(name="psACC", bufs=2, space="PSUM") as psACC,
            tc.tile_pool(name="psAUX", bufs=2, space="PSUM") as psAUX,
            tc.tile_pool(name="dram", bufs=1, space="DRAM") as dpool,
        ):
            # ---- phase 0: loads spread over 4 DMA rings ----------------
            # gpsimd ring: masks (cheap engine ops first), wqk, x chunks 1-7
            ones_sb = cpool.tile([128, 1], BF16)
            nc.gpsimd.memset(ones_sb[:], 1.0)
            tri_sb = cpool.tile([128, 128], BF16)
            nc.gpsimd.memset(tri_sb[:], 1.0)
            nc.gpsimd.affine_select(
                out=tri_sb[:],
                in_=tri_sb[:],
                compare_op=mybir.AluOpType.is_ge,
                fill=0.0,
                base=0,
                pattern=[[1, 128]],
                channel_multiplier=-1,
            )
            # selector for the destination-side denominator broadcast:
            # sel[k, c] = 1 iff floor(c/128) == k, so sel[:, g*128:(g+1)*128]
            # as lhsT broadcasts recip row g across all 128 out partitions
            sel_sb = cpool.tile([8, 8 * 128], BF16)
            nc.gpsimd.memset(sel_sb[:], 1.0)
            nc.gpsimd.affine_select(
                out=sel_sb[:],
                in_=sel_sb[:],
                compare_op=mybir.AluOpType.is_ge,
                fill=0.0,
                base=0,
                pattern=[[1, 8 * 128]],
                channel_multiplier=-128,
            )
            nc.gpsimd.affine_select(
                out=sel_sb[:],
                in_=sel_sb[:],
                compare_op=mybir.AluOpType.is_ge,
                fill=0.0,
                base=127,
                pattern=[[-1, 8 * 128]],
                channel_multiplier=128,
            )

            wqk_sb = wpool.tile([128, NCT, FQK], BF16, tag="w16", bufs=4)
            wqk_r = wqk.rearrange("(ct p) f -> p ct f", p=128)
            for pc in range(4):
                s = slice(pc * 4, (pc + 1) * 4)
                nc.gpsimd.dma_start(out=wqk_sb[:, s, :], in_=wqk_r[:, s, :])

            xt_tiles = {}
            # first x chunk on the sync ring, in 4 pieces (interleaved with
            # the small phase-1 constants), in parallel with wqk on gpsimd
            # so the first matmul can start asap
            xt_tiles[0] = wpool.tile(
                [128, NCT, 512], BF16, tag="xch", name="xT_ch0"
            )
            xT_r0 = xT[:, 0:512].rearrange("(ct p) t -> p ct t", p=128)
            for pc in range(2):
                s = slice(pc * 4, (pc + 1) * 4)
                nc.sync.dma_start(out=xt_tiles[0][:, s, :], in_=xT_r0[:, s, :])
            rmat_sb = cpool.tile([128, 128], BF16)
            nc.sync.dma_start(out=rmat_sb[:], in_=rmat[:])
            bqk_sb = cpool.tile([128, 4], F32)
            nc.sync.dma_start(out=bqk_sb[:], in_=bqk[:])
            cos_sb = cpool.tile([128, T], BF16)
            nc.sync.dma_start(out=cos_sb[:], in_=cosd[:])
            sin_sb = cpool.tile([128, T], BF16)
            nc.sync.dma_start(out=sin_sb[:], in_=sind[:])
            for pc in range(2, 4):
                s = slice(pc * 4, (pc + 1) * 4)
                nc.sync.dma_start(out=xt_tiles[0][:, s, :], in_=xT_r0[:, s, :])
            bv_sb = cpool.tile([128, FV], F32)
            nc.sync.dma_start(out=bv_sb[:], in_=bv[:])
            bproj_sb = cpool.tile([128, C], BF16)
            nc.sync.dma_start(out=bproj_sb[:], in_=bproj[:])
            # chunks 1..7 on gpsimd, one DMA each
            for ch in range(1, NT):
                t0 = ch * 512
                xt = wpool.tile(
                    [128, NCT, 512], BF16, tag="xch", name=f"xT_ch{ch}"
                )
                nc.gpsimd.dma_start(
                    out=xt[:],
                    in_=xT[:, t0 : t0 + 512].rearrange(
                        "(ct p) t -> p ct t", p=128
                    ),
                )
                xt_tiles[ch] = xt

            # scalar ring: wv then all of wproj (wp3 reuses wqk's slot, its
            # transfer is semaphore-gated until phase 1 stops reading wqk)
            wv_sb = wpool.tile([128, NCT, FV], BF16, tag="w8", bufs=2)
            nc.scalar.dma_start(
                out=wv_sb[:], in_=wv.rearrange("(ct p) f -> p ct f", p=128)
            )
            wp_tiles = []
            for ec in range(4):
                e0 = ec * 512
                wpt = wpool.tile(
                    [128, NCT, 512], BF16, tag="w16", bufs=4, name=f"wp{ec}"
                )
                nc.scalar.dma_start(
                    out=wpt[:],
                    in_=wproj[:, e0 : e0 + 512].rearrange(
                        "(ft p) e -> p ft e", p=128
                    ),
                )
                wp_tiles.append(wpt)

            # ---- phase 1: QKV projection + RoPE -----------------------
            # qkT[f, t] resident tiles (bf16): 4 m-tiles [128, TQ]
            qkT_sb = rpool.tile([128, 4, TQ], BF16)
            # v natural [t, f] resident: 32 token-tiles of [128, 256]
            v_sb = rpool.tile([128, TQ // 128, FV], BF16)

            for ch in range(NT):
                t0 = ch * 512
                tc0 = t0 % T  # position within cos/sin period
                xT_ch = xt_tiles[ch]
                for mi in range(4):
                    ps2 = psMM.tile([128, 2, 512], F32, tag="mm2")
                    ps = ps2[:, 0, :]
                    for ct in range(NCT):
                        nc.tensor.matmul(
                            ps,
                            lhsT=wqk_sb[:, ct, mi * 128 : (mi + 1) * 128],
                            rhs=xT_ch[:, ct, :],
                            start=(ct == 0),
                            stop=(ct == NCT - 1),
                        )
                    # evict + bias + RoPE; rotate-half via +-1 permutation
                    # matmul: dst = (ps+b)*cos + R^T @ ((ps+b)*sin)
                    m1 = wpool.tile([128, 512], BF16, tag="rope_m1")
                    m2 = wpool.tile([128, 512], BF16, tag="rope_m2")
                    nc.vector.scalar_tensor_tensor(
                        out=m2[:], in0=ps, scalar=bqk_sb[:, mi : mi + 1],
                        in1=sin_sb[:, tc0 : tc0 + 512], op0=add, op1=mult,
                    )
                    rot_ps = psACC.tile([128, 512], F32, tag="acc")
                    nc.tensor.matmul(
                        rot_ps[:], lhsT=rmat_sb[:], rhs=m2[:],
                        start=True, stop=True,
                    )
                    nc.vector.scalar_tensor_tensor(
                        out=m1[:], in0=ps, scalar=bqk_sb[:, mi : mi + 1],
                        in1=cos_sb[:, tc0 : tc0 + 512], op0=add, op1=mult,
                    )
                    dst = qkT_sb[:, mi, t0 : t0 + 512]
                    nc.vector.tensor_add(dst, m1[:], rot_ps[:])
                for tt in range(4):
                    psv = psACC.tile([128, 512], F32, tag="acc")
                    for ct in range(NCT):
                        nc.tensor.matmul(
                            psv[:, 0:FV],
                            lhsT=xT_ch[:, ct, tt * 128 : (tt + 1) * 128],
                            rhs=wv_sb[:, ct, :],
                            start=(ct == 0),
                            stop=(ct == NCT - 1),
                        )
                    nc.vector.tensor_add(
                        v_sb[:, ch * 4 + tt, :], psv[:, 0:FV], bv_sb[:]
                    )

            # ---- phase 2: attention, local head OUTERMOST -------------
            # a2a slot p (= b*2 + tqc) carries head hl's 128 unnormalized
            # yT rows + 1 denominator row for tq-window p, 512 tokens.
            a2a_in = [
                dpool.tile([NCORES, 129, 512], BF16, name=f"a2a_in{h}")
                for h in range(HPC)
            ]
            a2a_out = [
                dpool.tile([NCORES, 129, 512], BF16, name=f"a2a_out{h}")
                for h in range(HPC)
            ]

            for hl in range(HPC):
                qh = qkT_sb[:, hl, :]
                kh = qkT_sb[:, 2 + hl, :]
                for b in range(B):
                    for tqc in range(2):
                        tq0 = b * T + tqc * 512
                        nj = 4 * (tqc + 1)

                        def c0_of(j):
                            m = j - (nj - 4)
                            return 128 * m if m > 0 else 0

                        ot_ps = psACC.tile([128, 512], F32, tag="acc")
                        den_ps = psAUX.tile([1, 512], F32, tag="aux")
                        st_tiles = {}
                        pt_tiles = {}

                        def emit_scores(jp):
                            st2 = psMM.tile([128, 2, 512], F32, tag="mm2")
                            for jj in range(2):
                                j = 2 * jp + jj
                                c0 = c0_of(j)
                                s0 = b * T + j * 128
                                nc.tensor.matmul(
                                    st2[:, jj, c0:512],
                                    lhsT=kh[:, s0 : s0 + 128],
                                    rhs=qh[:, tq0 + c0 : tq0 + 512],
                                    start=True,
                                    stop=True,
                                )
                            st_tiles[jp] = st2

                        def emit_exp(jp):
                            st2 = st_tiles.pop(jp)
                            cu = c0_of(2 * jp)  # union (min) col offset
                            pt2 = apool.tile(
                                [128, 2, 512], BF16, tag="pt", bufs=3
                            )
                            nc.scalar.activation(
                                pt2[:, :, cu:512],
                                st2[:, :, cu:512],
                                Exp,
                                scale=SCALE,
                            )
                            pt_tiles[jp] = pt2

                        emit_scores(0)
                        if nj > 2:
                            emit_scores(1)
                        for jp in range(nj // 2):
                            emit_exp(jp)
                            if jp + 2 < nj // 2:
                                emit_scores(jp + 2)
                            pt2 = pt_tiles.pop(jp)
                            for jj in range(2):
                                j = 2 * jp + jj
                                c0 = c0_of(j)
                                m = j - (nj - 4)
                                if m >= 0:
                                    # mask the 128-wide diagonal triangle
                                    nc.vector.tensor_mul(
                                        pt2[:, jj, c0 : c0 + 128],
                                        pt2[:, jj, c0 : c0 + 128],
                                        tri_sb[:],
                                    )
                                pt = pt2[:, jj, c0:512]
                                vt = v_sb[
                                    :, b * 8 + j, hl * 128 : (hl + 1) * 128
                                ]
                                nc.tensor.matmul(
                                    ot_ps[:, c0:512], lhsT=vt, rhs=pt,
                                    start=(j == 0), stop=(j == nj - 1),
                                )
                                nc.tensor.matmul(
                                    den_ps[:, c0:512], lhsT=ones_sb[:],
                                    rhs=pt,
                                    start=(j == 0), stop=(j == nj - 1),
                                )
                        # unnormalized eviction; normalization happens at the
                        # destination core after the AllToAll
                        yt = apool.tile([128, 512], BF16, tag="yt")
                        nc.vector.tensor_scalar_mul(yt[:], ot_ps[:], 1.0)
                        dn = apool.tile([1, 512], BF16, tag="dn")
                        nc.vector.tensor_scalar_mul(dn[:], den_ps[:], 1.0)
                        p = b * 2 + tqc
                        nc.sync.dma_start(
                            out=a2a_in[hl][p, 0:128, :], in_=yt[:]
                        )
                        nc.sync.dma_start(
                            out=a2a_in[hl][p, 128:129, :], in_=dn[:]
                        )
                if hl == 0:
                    # launch a2a#0 under head 1's compute
                    nc.gpsimd.collective_compute(
                        "AllToAll",
                        mybir.AluOpType.bypass,
                        replica_groups=[list(range(NCORES))],
                        ins=[a2a_in[0][:].opt()],
                        outs=[a2a_out[0][:].opt()],
                    )

            nc.gpsimd.collective_compute(
                "AllToAll",
                mybir.AluOpType.bypass,
                replica_groups=[list(range(NCORES))],
                ins=[a2a_in[1][:].opt()],
                outs=[a2a_out[1][:].opt()],
            )

            # ---- phase 3: destination-side normalize + projection -----
            # pass h contracts the 8 feature blocks of global heads
            # {2g + h}; bias folded into pass 0's bf16 partials.
            yts = []
            part_sb = wpool.tile([128, 16, 512], BF16, tag="xch", name="part")
            for h in range(HPC):
                yts_sb = wpool.tile(
                    [128, 8, 512], BF16, tag="w8", bufs=2, name=f"yts{h}"
                )
                nc.sync.dma_start(
                    out=yts_sb[:],
                    in_=a2a_out[h][:, 0:128, :].rearrange("g p t -> p g t"),
                )
                den_sb = apool.tile([8, 512], F32, tag="den8", name=f"den{h}")
                nc.gpsimd.dma_start(
                    out=den_sb[:],
                    in_=a2a_out[h][:, 128:129, :].rearrange(
                        "g o t -> (g o) t"
                    ),
                )
                recip = apool.tile([8, 512], F32, tag="recip8")
                nc.vector.reciprocal_approx_fast(recip[:], den_sb[:])
                recip_bf = apool.tile([8, 512], BF16, tag="recipbf")
                nc.vector.tensor_scalar_mul(recip_bf[:], recip[:], 1.0)
                for g in range(NCORES):
                    rb_ps = psACC.tile([128, 512], F32, tag="acc")
                    nc.tensor.matmul(
                        rb_ps[:],
                        lhsT=sel_sb[:, g * 128 : (g + 1) * 128],
                        rhs=recip_bf[:],
                        start=True,
                        stop=True,
                    )
                    nc.vector.tensor_mul(
                        yts_sb[:, g, :], yts_sb[:, g, :], rb_ps[:]
                    )
                yts.append(yts_sb)

                for ec in range(4):
                    e0 = ec * 512
                    wp_sb = wp_tiles[ec]
                    for tt in range(4):
                        tl = tt * 128
                        pps2 = psMM.tile([128, 2, 512], F32, tag="mm2")
                        pps = pps2[:, 0, :]
                        for g in range(NCORES):
                            nc.tensor.matmul(
                                pps,
                                lhsT=yts_sb[:, g, tl : tl + 128],
                                rhs=wp_sb[:, 2 * g + h, :],
                                start=(g == 0),
                                stop=(g == NCORES - 1),
                            )
                        if h == 0:
                            nc.vector.tensor_add(
                                part_sb[:, ec * 4 + tt, :],
                                pps,
                                bproj_sb[:, e0 : e0 + 512],
                            )
                        else:
                            fin = apool.tile([128, 512], F32, tag="fin")
                            nc.vector.tensor_add(
                                fin[:], pps, part_sb[:, ec * 4 + tt, :]
                            )
                            nc.sync.dma_start(
                                out=out[tl : tl + 128, e0 : e0 + 512],
                                in_=fin[:],
                            )

    nc.compile()
    return nc


def _rope_tables():
    inv = 1.0 / (10000.0 ** (np.arange(0, D, 2, dtype=np.float64) / D))
    t = np.arange(T, dtype=np.float64)
    fr = np.outer(t, inv)  # [T, 64]
    cosT = np.tile(np.cos(fr).T, (2, 1)).astype(ml_dtypes.bfloat16)
    sinT = np.tile(np.sin(fr).T, (2, 1)).astype(ml_dtypes.bfloat16)
    return np.ascontiguousarray(cosT), np.ascontiguousarray(sinT)


def _prep_inputs(x, Wqkv, bqkv, Wproj, bproj):
    bf = ml_dtypes.bfloat16
    x = np.asarray(x, np.float32).reshape(TQ, C)
    Wqkv = np.asarray(Wqkv, np.float32)
    bqkv = np.asarray(bqkv, np.float32)
    Wproj = np.asarray(Wproj, np.float32)
    bproj = np.asarray(bproj, np.float32)

    xT = np.ascontiguousarray(x.T.astype(bf))
    cosT, sinT = _rope_tables()
    rmat = np.zeros((128, 128), bf)
    for i in range(64):
        rmat[64 + i, i] = -1.0   # out[p<64]  = -m2[p+64]
        rmat[i, 64 + i] = 1.0    # out[p>=64] = +m2[p-64]
    wproj_b = np.ascontiguousarray(Wproj.astype(bf))
    bproj_b = np.ascontiguousarray(
        np.broadcast_to(bproj[None, :].astype(bf), (128, C))
    )

    Wq = Wqkv[:, 0 * C : 1 * C].reshape(C, H, D)
    Wk = Wqkv[:, 1 * C : 2 * C].reshape(C, H, D)
    Wv = Wqkv[:, 2 * C : 3 * C].reshape(C, H, D)
    bq = bqkv[0 * C : 1 * C].reshape(H, D)
    bk = bqkv[1 * C : 2 * C].reshape(H, D)
    bvv = bqkv[2 * C : 3 * C].reshape(H, D)

    in_maps = []
    for r in range(NCORES):
        ha, hb = 2 * r, 2 * r + 1
        wqk_s = np.ascontiguousarray(
            np.concatenate(
                [Wq[:, ha], Wq[:, hb], Wk[:, ha], Wk[:, hb]], axis=1
            ).astype(bf)
        )
        bqk_s = np.ascontiguousarray(
            np.stack([bq[ha], bq[hb], bk[ha], bk[hb]], axis=1)
        )  # [128, 4]
        wv_s = np.ascontiguousarray(
            np.concatenate([Wv[:, ha], Wv[:, hb]], axis=1).astype(bf)
        )
        bv_s = np.ascontiguousarray(
            np.broadcast_to(
                np.concatenate([bvv[ha], bvv[hb]])[None, :], (128, FV)
            )
        )
        in_maps.append(
            {
                "xT": xT,
                "wqk": wqk_s,
                "wv": wv_s,
                "bqk": bqk_s,
                "bv": bv_s,
                "wproj": wproj_b,
                "bproj": bproj_b,
                "cosd": cosT,
                "sind": sinT,
                "rmat": rmat,
            }
        )
    return in_maps


def kernel(x, Wqkv, bqkv, Wproj, bproj, _trace=False, _trace_kwargs=None):
    if "nc" not in _CACHE:
        _CACHE["nc"] = _build_program()
    nc = _CACHE["nc"]
    in_maps = _prep_inputs(x, Wqkv, bqkv, Wproj, bproj)
    kwargs = {}
    if _trace:
        kwargs.update(trace=True, **(_trace_kwargs or {}))
    res = run_bass_kernel_spmd(nc, in_maps, core_ids=list(range(NCORES)), **kwargs)
    _CACHE["last_results"] = res
    out = np.concatenate([res.results[r]["out"] for r in range(NCORES)], axis=0)
    return np.ascontiguousarray(out.reshape(B, T, C).astype(np.float32))


# revision 11
# speedup vs baseline: 1.0138x; 1.0138x over previous
"""Trainium2 Bass kernel for causal self-attention with RoPE (Megatron-style
head-parallel over 8 NeuronCores).

Sharding: 16 heads / 8 cores = 2 heads per core. Wqkv is split column-wise by
head (each core computes q/k/v for its 2 heads for the full batch); attention
is embarrassingly parallel over (batch, head); the output projection is
row-parallel with the partial contraction exchanged via two AllToAlls (one per
local head) so that core r ends up owning output rows [r*512, (r+1)*512) of
the flattened [4096, 2048] output, which the host concatenates.

Key scheduling ideas vs the naive version:
 - all inputs are pre-converted to bf16 on the host (halves HBM traffic);
   first x chunk + first weight pieces ride separate DMA rings so the PE
   starts within a few us.
 - attention loops local-head OUTERMOST: after head 0 of every (b, tq-chunk)
   is done, its AllToAll launches and hides under head 1's compute; the
   second AllToAll hides under projection pass 1 (even global heads).
 - softmax normalization happens at the DESTINATION core: each a2a slot
   carries 128 unnormalized y rows + 1 denominator row; the receiver does
   reciprocal + partition-broadcast + per-head scaling before the projection.
 - causal structure is exploited at 128-column granularity: diagonal key
   blocks only compute score/av/den columns c >= 128*m (ascending-j PSUM
   accumulation), and the triangular mask shrinks to one [128,128] multiply.
 - the projection contracts in two passes (even heads, then odd heads after
   a2a#1) with bf16 partials; bias is folded into pass 1.

All matmuls run in bf16 with fp32 PSUM accumulation. Softmax skips the
max-subtraction (scores are O(+-10) here, exp stays in range). RoPE's
rotate-half runs as a constant +-1 permutation matmul on the PE.
"""

import sys

if "/opt/trn_rl_repo" not in sys.path:
    sys.path.insert(0, "/opt/trn_rl_repo")

import ml_dtypes
import numpy as np

import concourse.bacc as bacc
import concourse.bass as bass
import concourse.mybir as mybir
import concourse.tile as tile
from concourse.bass_utils import run_bass_kernel_spmd

B, T, C, H, D = 4, 1024, 2048, 16, 128
TQ = B * T           # 4096 flattened tokens
NCORES = 8
HPC = H // NCORES    # heads per core = 2
FQK = 4 * D          # 512 qkT feature rows per core (qa, qb, ka, kb)
FV = HPC * D         # 256 v feature cols per core
ROWS = TQ // NCORES  # 512 output rows per core
NCT = C // 128       # 16 contraction tiles
SCALE = 1.0 / float(np.sqrt(D))

F32 = mybir.dt.float32
BF16 = mybir.dt.bfloat16

_CACHE = {}


def _build_program():
    nc = bacc.Bacc(
        "TRN2",
        target_bir_lowering=False,
        debug=False,
        enable_asserts=False,
        num_devices=NCORES,
    )

    # ---- I/O (all big tensors pre-converted to bf16 on the host) -------
    xT = nc.dram_tensor("xT", [C, TQ], BF16, kind="ExternalInput")
    wqk = nc.dram_tensor("wqk", [C, FQK], BF16, kind="ExternalInput")
    wv = nc.dram_tensor("wv", [C, FV], BF16, kind="ExternalInput")
    bqk = nc.dram_tensor("bqk", [128, 4], F32, kind="ExternalInput")
    bv = nc.dram_tensor("bv", [128, FV], F32, kind="ExternalInput")
    wproj = nc.dram_tensor("wproj", [C, C], BF16, kind="ExternalInput")
    bproj = nc.dram_tensor("bproj", [128, C], BF16, kind="ExternalInput")
    cosd = nc.dram_tensor("cosd", [128, T], BF16, kind="ExternalInput")
    sind = nc.dram_tensor("sind", [128, T], BF16, kind="ExternalInput")
    rmat = nc.dram_tensor("rmat", [128, 128], BF16, kind="ExternalInput")
    out = nc.dram_tensor("out", [ROWS, C], F32, kind="ExternalOutput")

    NT = TQ // 512  # 8 token chunks of 512
    Exp = mybir.ActivationFunctionType.Exp
    add = mybir.AluOpType.add
    mult = mybir.AluOpType.mult

    with tile.TileContext(nc) as tc:
        with (
            tc.tile_pool(name="const", bufs=1) as cpool,
            tc.tile_pool(name="resident", bufs=1) as rpool,
            tc.tile_pool(name="work", bufs=2) as wpool,
            tc.tile_pool(name="att", bufs=2) as apool,
            tc.tile_pool(name="psMM", bufs=2, space="PSUM") as psMM,
            tc.tile_pool(name="psACC", bufs=2, space="PSUM") as psACC,
            tc.tile_pool(name="psAUX", bufs=2, space="PSUM") as psAUX,
            tc.tile_pool(name="dram", bufs=1, space="DRAM") as dpool,
        ):
            # ---- phase 0: loads spread over 3 DMA rings ----------------
            wqk_sb = wpool.tile([128, NCT, FQK], BF16, tag="w16", bufs=4)
            wqk_r = wqk.rearrange("(ct p) f -> p ct f", p=128)
            for pc in range(4):
                s = slice(pc * 4, (pc + 1) * 4)
                nc.gpsimd.dma_start(out=wqk_sb[:, s, :], in_=wqk_r[:, s, :])

            xt_tiles = {}
            # first x chunk on the sync ring, in 4 pieces (interleaved with
            # the small phase-1 constants), in parallel with wqk on gpsimd
            # so the first matmul can start asap
            xt_tiles[0] = wpool.tile(
                [128, NCT, 512], BF16, tag="xch", name="xT_ch0"
            )
            xT_r0 = xT[:, 0:512].rearrange("(ct p) t -> p ct t", p=128)
            for pc in range(2):
                s = slice(pc * 4, (pc + 1) * 4)
                nc.sync.dma_start(out=xt_tiles[0][:, s, :], in_=xT_r0[:, s, :])
            rmat_sb = cpool.tile([128, 128], BF16)
            nc.sync.dma_start(out=rmat_sb[:], in_=rmat[:])
            bqk_sb = cpool.tile([128, 4], F32)
            nc.sync.dma_start(out=bqk_sb[:], in_=bqk[:])
            cos_sb = cpool.tile([128, T], BF16)
            nc.sync.dma_start(out=cos_sb[:], in_=cosd[:])
            sin_sb = cpool.tile([128, T], BF16)
            nc.sync.dma_start(out=sin_sb[:], in_=sind[:])
            for pc in range(2, 4):
                s = slice(pc * 4, (pc + 1) * 4)
                nc.sync.dma_start(out=xt_tiles[0][:, s, :], in_=xT_r0[:, s, :])
            bv_sb = cpool.tile([128, FV], F32)
            nc.sync.dma_start(out=bv_sb[:], in_=bv[:])
            bproj_sb = cpool.tile([128, C], BF16)
            nc.sync.dma_start(out=bproj_sb[:], in_=bproj[:])
            # chunks 1..7 on gpsimd, one DMA each
            for ch in range(1, NT):
                t0 = ch * 512
                xt = wpool.tile(
                    [128, NCT, 512], BF16, tag="xch", name=f"xT_ch{ch}"
                )
                nc.gpsimd.dma_start(
                    out=xt[:],
                    in_=xT[:, t0 : t0 + 512].rearrange(
                        "(ct p) t -> p ct t", p=128
                    ),
                )
                xt_tiles[ch] = xt

            # scalar ring: wv now; wproj is emitted later (mid phase 1) so
            # its 4MB does not starve the early x-chunk loads
            wv_sb = wpool.tile([128, NCT, FV], BF16, tag="w8", bufs=2)
            nc.scalar.dma_start(
                out=wv_sb[:], in_=wv.rearrange("(ct p) f -> p ct f", p=128)
            )

            # warm up the collective stream early so a2a#0 does not pay the
            # ~11us first-collective trigger latency
            cc_warm_in = dpool.tile([NCORES, 1, 64], BF16, name="cc_warm_in")
            cc_warm_out = dpool.tile([NCORES, 1, 64], BF16, name="cc_warm_out")
            nc.gpsimd.collective_compute(
                "AllToAll",
                mybir.AluOpType.bypass,
                replica_groups=[list(range(NCORES))],
                ins=[cc_warm_in[:].opt()],
                outs=[cc_warm_out[:].opt()],
            )

            # phase-2/3 constants (emitted after the DMA descriptors: the
            # gpsimd engine is free once descriptors are generated)
            ones_sb = cpool.tile([128, 1], BF16)
            nc.gpsimd.memset(ones_sb[:], 1.0)
            tri_sb = cpool.tile([128, 128], BF16)
            nc.gpsimd.memset(tri_sb[:], 1.0)
            nc.gpsimd.affine_select(
                out=tri_sb[:],
                in_=tri_sb[:],
                compare_op=mybir.AluOpType.is_ge,
                fill=0.0,
                base=0,
                pattern=[[1, 128]],
                channel_multiplier=-1,
            )
            # selector for the destination-side denominator broadcast:
            # sel[k, c] = 1 iff floor(c/128) == k, so sel[:, g*128:(g+1)*128]
            # as lhsT broadcasts recip row g across all 128 out partitions
            sel_sb = cpool.tile([8, 8 * 128], BF16)
            nc.gpsimd.memset(sel_sb[:], 1.0)
            nc.gpsimd.affine_select(
                out=sel_sb[:],
                in_=sel_sb[:],
                compare_op=mybir.AluOpType.is_ge,
                fill=0.0,
                base=0,
                pattern=[[1, 8 * 128]],
                channel_multiplier=-128,
            )
            nc.gpsimd.affine_select(
                out=sel_sb[:],
                in_=sel_sb[:],
                compare_op=mybir.AluOpType.is_ge,
                fill=0.0,
                base=127,
                pattern=[[-1, 8 * 128]],
                channel_multiplier=128,
            )
            wp_tiles = []

            # ---- phase 1: QKV projection + RoPE -----------------------
            # qkT[f, t] resident tiles (bf16): 4 m-tiles [128, TQ]
            qkT_sb = rpool.tile([128, 4, TQ], BF16)
            # v natural [t, f] resident: 32 token-tiles of [128, 256]
            v_sb = rpool.tile([128, TQ // 128, FV], BF16)

            for ch in range(NT):
                t0 = ch * 512
                tc0 = t0 % T  # position within cos/sin period
                xT_ch = xt_tiles[ch]
                for mi in range(4):
                    ps2 = psMM.tile([128, 2, 512], F32, tag="mm2")
                    ps = ps2[:, 0, :]
                    for ct in range(NCT):
                        nc.tensor.matmul(
                            ps,
                            lhsT=wqk_sb[:, ct, mi * 128 : (mi + 1) * 128],
                            rhs=xT_ch[:, ct, :],
                            start=(ct == 0),
                            stop=(ct == NCT - 1),
                        )
                    # evict + bias + RoPE; rotate-half via +-1 permutation
                    # matmul: dst = (ps+b)*cos + R^T @ ((ps+b)*sin)
                    m1 = wpool.tile([128, 512], BF16, tag="rope_m1")
                    m2 = wpool.tile([128, 512], BF16, tag="rope_m2")
                    nc.vector.scalar_tensor_tensor(
                        out=m2[:], in0=ps, scalar=bqk_sb[:, mi : mi + 1],
                        in1=sin_sb[:, tc0 : tc0 + 512], op0=add, op1=mult,
                    )
                    rot_ps = psACC.tile([128, 512], F32, tag="acc")
                    nc.tensor.matmul(
                        rot_ps[:], lhsT=rmat_sb[:], rhs=m2[:],
                        start=True, stop=True,
                    )
                    nc.vector.scalar_tensor_tensor(
                        out=m1[:], in0=ps, scalar=bqk_sb[:, mi : mi + 1],
                        in1=cos_sb[:, tc0 : tc0 + 512], op0=add, op1=mult,
                    )
                    dst = qkT_sb[:, mi, t0 : t0 + 512]
                    nc.vector.tensor_add(dst, m1[:], rot_ps[:])
                for tt in range(4):
                    psv = psACC.tile([128, 512], F32, tag="acc")
                    for ct in range(NCT):
                        nc.tensor.matmul(
                            psv[:, 0:FV],
                            lhsT=xT_ch[:, ct, tt * 128 : (tt + 1) * 128],
                            rhs=wv_sb[:, ct, :],
                            start=(ct == 0),
                            stop=(ct == NCT - 1),
                        )
                    nc.vector.tensor_add(
                        v_sb[:, ch * 4 + tt, :], psv[:, 0:FV], bv_sb[:]
                    )
                if ch == 2:
                    # wproj loads, emitted once the early x loads are clear
                    for ec in range(4):
                        e0 = ec * 512
                        wpt = wpool.tile(
                            [128, NCT, 512], BF16, tag="w16", bufs=4,
                            name=f"wp{ec}",
                        )
                        nc.scalar.dma_start(
                            out=wpt[:],
                            in_=wproj[:, e0 : e0 + 512].rearrange(
                                "(ft p) e -> p ft e", p=128
                            ),
                        )
                        wp_tiles.append(wpt)

            # ---- phase 2: attention, local head OUTERMOST -------------
            # a2a slot p (= b*2 + tqc) carries head hl's 128 unnormalized
            # yT rows + 1 denominator row for tq-window p, 512 tokens.
            a2a_in = [
                dpool.tile([NCORES, 129, 512], BF16, name=f"a2a_in{h}")
                for h in range(HPC)
            ]
            a2a_out = [
                dpool.tile([NCORES, 129, 512], BF16, name=f"a2a_out{h}")
                for h in range(HPC)
            ]

            den_tiles = []
            for hl in range(HPC):
                qh = qkT_sb[:, hl, :]
                kh = qkT_sb[:, 2 + hl, :]
                for b in range(B):
                    for tqc in range(2):
                        tq0 = b * T + tqc * 512
                        nj = 4 * (tqc + 1)

                        def c0_of(j):
                            m = j - (nj - 4)
                            return 128 * m if m > 0 else 0

                        ot_ps = psACC.tile([128, 512], F32, tag="acc")
                        den_ps = psAUX.tile([1, 512], F32, tag="aux")
                        pts = {}

                        def emit_st(jp):
                            st2 = psMM.tile([128, 2, 512], F32, tag="mm2")
                            for jj in range(2):
                                j = 2 * jp + jj
                                c0 = c0_of(j)
                                s0 = b * T + j * 128
                                nc.tensor.matmul(
                                    st2[:, jj, c0:512],
                                    lhsT=kh[:, s0 : s0 + 128],
                                    rhs=qh[:, tq0 + c0 : tq0 + 512],
                                    start=True,
                                    stop=True,
                                )
                            return st2

                        def emit_exp(jp, st2):
                            cu = c0_of(2 * jp)  # union (min) col offset
                            pt2 = apool.tile(
                                [128, 2, 512], BF16, tag="pt", bufs=6
                            )
                            nc.scalar.activation(
                                pt2[:, :, cu:512],
                                st2[:, :, cu:512],
                                Exp,
                                scale=SCALE,
                            )
                            for jj in range(2):
                                j = 2 * jp + jj
                                c0 = c0_of(j)
                                if j - (nj - 4) >= 0:
                                    # mask the 128-wide diagonal triangle
                                    nc.vector.tensor_mul(
                                        pt2[:, jj, c0 : c0 + 128],
                                        pt2[:, jj, c0 : c0 + 128],
                                        tri_sb[:],
                                    )
                            pts[jp] = pt2

                        # score run (consecutive on PE), exps pipelined
                        st_prev = emit_st(0)
                        for jp in range(1, nj // 2):
                            st_cur = emit_st(jp)
                            emit_exp(jp - 1, st_prev)
                            st_prev = st_cur
                        emit_exp(nj // 2 - 1, st_prev)
                        # attention @ V: one uninterrupted accumulation run
                        for j in range(nj):
                            c0 = c0_of(j)
                            nc.tensor.matmul(
                                ot_ps[:, c0:512],
                                lhsT=v_sb[
                                    :, b * 8 + j, hl * 128 : (hl + 1) * 128
                                ],
                                rhs=pts[j // 2][:, j % 2, c0:512],
                                start=(j == 0),
                                stop=(j == nj - 1),
                            )
                        # denominator: second uninterrupted run
                        for j in range(nj):
                            c0 = c0_of(j)
                            nc.tensor.matmul(
                                den_ps[:, c0:512],
                                lhsT=ones_sb[:],
                                rhs=pts[j // 2][:, j % 2, c0:512],
                                start=(j == 0),
                                stop=(j == nj - 1),
                            )
                        # unnormalized eviction; normalization happens at the
                        # destination core after the AllToAll
                        yt = apool.tile([128, 512], BF16, tag="yt")
                        nc.vector.tensor_scalar_mul(yt[:], ot_ps[:], 1.0)
                        dn = apool.tile([1, 512], BF16, tag="dn")
                        nc.vector.tensor_scalar_mul(dn[:], den_ps[:], 1.0)
                        p = b * 2 + tqc
                        nc.sync.dma_start(
                            out=a2a_in[hl][p, 0:128, :], in_=yt[:]
                        )
                        nc.sync.dma_start(
                            out=a2a_in[hl][p, 128:129, :], in_=dn[:]
                        )
                # launch a2a#hl (hl=0 hides under head 1's compute, hl=1
                # under projection pass 0); the denominator rows come back
                # on the gpsimd ring right after each collective
                nc.gpsimd.collective_compute(
                    "AllToAll",
                    mybir.AluOpType.bypass,
                    replica_groups=[list(range(NCORES))],
                    ins=[a2a_in[hl][:].opt()],
                    outs=[a2a_out[hl][:].opt()],
                )
                den_sb = apool.tile(
                    [8, 512], F32, tag="den8", name=f"den{hl}"
                )
                nc.gpsimd.dma_start(
                    out=den_sb[:],
                    in_=a2a_out[hl][:, 128:129, :].rearrange(
                        "g o t -> (g o) t"
                    ),
                )
                den_tiles.append(den_sb)

            # ---- phase 3: destination-side normalize + projection -----
            # pass h contracts the 8 feature blocks of global heads
            # {2g + h}; bias folded into pass 0's bf16 partials.
            yts = []
            part_sb = wpool.tile([128, 16, 512], BF16, tag="xch", name="part")
            for h in range(HPC):
                yts_sb = wpool.tile(
                    [128, 8, 512], BF16, tag="w8", bufs=2, name=f"yts{h}"
                )
                nc.sync.dma_start(
                    out=yts_sb[:],
                    in_=a2a_out[h][:, 0:128, :].rearrange("g p t -> p g t"),
                )
                recip = apool.tile([8, 512], F32, tag="recip8")
                nc.vector.reciprocal_approx_fast(recip[:], den_tiles[h][:])
                recip_bf = apool.tile([8, 512], BF16, tag="recipbf")
                nc.vector.tensor_scalar_mul(recip_bf[:], recip[:], 1.0)
                for g in range(NCORES):
                    rb_ps = psACC.tile([128, 512], F32, tag="acc")
                    nc.tensor.matmul(
                        rb_ps[:],
                        lhsT=sel_sb[:, g * 128 : (g + 1) * 128],
                        rhs=recip_bf[:],
                        start=True,
                        stop=True,
                    )
                    nc.vector.tensor_mul(
                        yts_sb[:, g, :], yts_sb[:, g, :], rb_ps[:]
                    )
                yts.append(yts_sb)

                for ec in range(4):
                    e0 = ec * 512
                    wp_sb = wp_tiles[ec]
                    for tt in range(4):
                        tl = tt * 128
                        pps2 = psMM.tile([128, 2, 512], F32, tag="mm2")
                        pps = pps2[:, 0, :]
                        for g in range(NCORES):
                            nc.tensor.matmul(
                                pps,
                                lhsT=yts_sb[:, g, tl : tl + 128],
                                rhs=wp_sb[:, 2 * g + h, :],
                                start=(g == 0),
                                stop=(g == NCORES - 1),
                            )
                        if h == 0:
                            nc.vector.tensor_add(
                                part_sb[:, ec * 4 + tt, :],
                                pps,
                                bproj_sb[:, e0 : e0 + 512],
                            )
                        else:
                            fin = apool.tile([128, 512], F32, tag="fin")
                            nc.vector.tensor_add(
                                fin[:], pps, part_sb[:, ec * 4 + tt, :]
                            )
                            nc.sync.dma_start(
                                out=out[tl : tl + 128, e0 : e0 + 512],
                                in_=fin[:],
                            )

    nc.compile()
    return nc


def _rope_tables():
    inv = 1.0 / (10000.0 ** (np.arange(0, D, 2, dtype=np.float64) / D))
    t = np.arange(T, dtype=np.float64)
    fr = np.outer(t, inv)  # [T, 64]
    cosT = np.tile(np.cos(fr).T, (2, 1)).astype(ml_dtypes.bfloat16)
    sinT = np.tile(np.sin(fr).T, (2, 1)).astype(ml_dtypes.bfloat16)
    return np.ascontiguousarray(cosT), np.ascontiguousarray(sinT)


def _prep_inputs(x, Wqkv, bqkv, Wproj, bproj):
    bf = ml_dtypes.bfloat16
    x = np.asarray(x, np.float32).reshape(TQ, C)
    Wqkv = np.asarray(Wqkv, np.float32)
    bqkv = np.asarray(bqkv, np.float32)
    Wproj = np.asarray(Wproj, np.float32)
    bproj = np.asarray(bproj, np.float32)

    xT = np.ascontiguousarray(x.T.astype(bf))
    cosT, sinT = _rope_tables()
    rmat = np.zeros((128, 128), bf)
    for i in range(64):
        rmat[64 + i, i] = -1.0   # out[p<64]  = -m2[p+64]
        rmat[i, 64 + i] = 1.0    # out[p>=64] = +m2[p-64]
    wproj_b = np.ascontiguousarray(Wproj.astype(bf))
    bproj_b = np.ascontiguousarray(
        np.broadcast_to(bproj[None, :].astype(bf), (128, C))
    )

    Wq = Wqkv[:, 0 * C : 1 * C].reshape(C, H, D)
    Wk = Wqkv[:, 1 * C : 2 * C].reshape(C, H, D)
    Wv = Wqkv[:, 2 * C : 3 * C].reshape(C, H, D)
    bq = bqkv[0 * C : 1 * C].reshape(H, D)
    bk = bqkv[1 * C : 2 * C].reshape(H, D)
    bvv = bqkv[2 * C : 3 * C].reshape(H, D)

    in_maps = []
    for r in range(NCORES):
        ha, hb = 2 * r, 2 * r + 1
        wqk_s = np.ascontiguousarray(
            np.concatenate(
                [Wq[:, ha], Wq[:, hb], Wk[:, ha], Wk[:, hb]], axis=1
            ).astype(bf)
        )
        bqk_s = np.ascontiguousarray(
            np.stack([bq[ha], bq[hb], bk[ha], bk[hb]], axis=1)
        )  # [128, 4]
        wv_s = np.ascontiguousarray(
            np.concatenate([Wv[:, ha], Wv[:, hb]], axis=1).astype(bf)
        )
        bv_s = np.ascontiguousarray(
            np.broadcast_to(
                np.concatenate([bvv[ha], bvv[hb]])[None, :], (128, FV)
            )
        )
        in_maps.append(
            {
                "xT": xT,
                "wqk": wqk_s,
                "wv": wv_s,
                "bqk": bqk_s,
                "bv": bv_s,
                "wproj": wproj_b,
                "bproj": bproj_b,
                "cosd": cosT,
                "sind": sinT,
                "rmat": rmat,
            }
        )
    return in_maps


def kernel(x, Wqkv, bqkv, Wproj, bproj, _trace=False, _trace_kwargs=None):
    if "nc" not in _CACHE:
        _CACHE["nc"] = _build_program()
    nc = _CACHE["nc"]
    in_maps = _prep_inputs(x, Wqkv, bqkv, Wproj, bproj)
    kwargs = {}
    if _trace:
        kwargs.update(trace=True, **(_trace_kwargs or {}))
    res = run_bass_kernel_spmd(nc, in_maps, core_ids=list(range(NCORES)), **kwargs)
    _CACHE["last_results"] = res
    out = np.concatenate([res.results[r]["out"] for r in range(NCORES)], axis=0)
    return np.ascontiguousarray(out.reshape(B, T, C).astype(np.float32))
